# revision 3
# baseline (speedup 1.0000x reference)
"""Trainium2 Bass kernel for DeepSeek-style MoE gate routing (v3, scheme T).

hidden_states [8, 4096, 2048] f32, w [256, 2048] f32, bias [256] f32
 -> topk_idx [32768, 8] int32, topk_weight [32768, 8] f32

Tokens split 8 ways across cores (4096/core).  Matmul runs TRANSPOSED
(w stationary, tokens moving) so the gate weight is the reused stationary
operand:

  main:  lg[eh] = sum_k w16[k,eh].T @ x16[k]          fp16, N<=512
  corr:  one fp8 DoubleRow matmul per k computes BOTH correction
         products (xl*w + x*wl) as the two DR k-tiles, scale 2^18
  fold:  logits = main + corr * 2^-18                 (DVE)
  T:     PE-transpose 128x128 fp32 blocks back to [token, expert]
  then sigmoid (ACT) + grouped top-k (DVE) as usual.

Effective logit precision ~1e-5 (x and w at ~16 bits), matching fp32-
grade routing; PE cost is 2 cyc/row + transposes vs 3 cyc/row for the
bf16 split3 scheme.

Outputs: top-8 indices (u16) and raw top-8 score values; the bias-
subtract + normalize epilogue runs on host over [T, 8].

Self-contained: hardcodes all shapes; only imports the concourse toolchain.
"""
import sys

if "/opt/trn_rl_repo" not in sys.path:
    sys.path.insert(0, "/opt/trn_rl_repo")

import numpy as np

import concourse.bass as bass  # noqa: F401
import concourse.mybir as mybir
import concourse.tile as tile
from concourse import bacc
from concourse.bass_utils import run_bass_kernel_spmd

P = 128
H = 2048
E = 256
KO = H // P        # 16
N_CORES = 8
T_CORE = 4096
N_TILES = T_CORE // P       # 32

N_GROUP = 8
GSIZE = E // N_GROUP
TOPK_GROUP = 4
TOP_K = 8
SCALING = 2.5
NEG_BIG = -1.0e30

MATMUL_MODE = "schemeT"     # kept for test harness compatibility

SC_CORR = 2.0 ** 18         # fp8 correction scale: w8s=w*2^5, xl8s=xl*2^13,
                            # wl8s=wl*2^18, x8=x
N_WARM = 18

# chunks of token tiles (128 tokens each): small start for fast PE rampup,
# small end for a short serial tail
CHUNKS = [1, 2, 4, 6, 6, 6, 4, 2, 1]
assert sum(CHUNKS) == N_TILES

f32 = mybir.dt.float32
f16 = mybir.dt.float16
fp8 = mybir.dt.float8e4
u16 = mybir.dt.uint16
ALU = mybir.AluOpType
ACTF = mybir.ActivationFunctionType
AX = mybir.AxisListType
PM = mybir.MatmulPerfMode

_CACHED_NC = {}


def _tb_blocks(ct):
    """split a chunk of ct tiles into matmul token-blocks of <=4 tiles"""
    out = []
    t0 = 0
    while t0 < ct:
        n = min(4, ct - t0)
        out.append((t0, n))
        t0 += n
    return out


def build_kernel(mode=MATMUL_MODE):
    assert mode == "schemeT"
    nc = bacc.Bacc("TRN2", target_bir_lowering=False, debug=False)

    d_x16 = nc.dram_tensor("x16p", [KO * P * T_CORE], f16, kind="ExternalInput")
    d_xdr = nc.dram_tensor("xdrp", [KO * P * 2 * T_CORE], fp8, kind="ExternalInput")
    d_w16 = nc.dram_tensor("w16p", [P, KO, 2, P], f16, kind="ExternalInput")
    d_wdr = nc.dram_tensor("wdrp", [P, KO, 2, 2, P], fp8, kind="ExternalInput")
    d_bias = nc.dram_tensor("biasrep", [P, E], f32, kind="ExternalInput")
    d_ident = nc.dram_tensor("ident", [P, P], f32, kind="ExternalInput")
    d_oidx = nc.dram_tensor("oidx", [P, N_TILES, TOP_K], u16, kind="ExternalOutput")
    d_owgt = nc.dram_tensor("owgt", [P, N_TILES, TOP_K], f32, kind="ExternalOutput")

    with tile.TileContext(nc) as tc:
        with tc.tile_pool(name="const", bufs=1) as cpool, \
             tc.tile_pool(name="xin", bufs=2) as xpool, \
             tc.tile_pool(name="lgp", bufs=6) as lpool, \
             tc.tile_pool(name="score", bufs=2) as spool, \
             tc.tile_pool(name="small", bufs=3) as mpool, \
             tc.tile_pool(name="psum", bufs=2, space="PSUM") as ppool, \
             tc.tile_pool(name="psumt", bufs=3, space="PSUM") as tpool, \
             tc.tile_pool(name="psumw", bufs=1, space="PSUM") as wpool:

            w16sb = cpool.tile([P, KO, 2, P], f16)
            nc.sync.dma_start(w16sb, d_w16.ap())
            wdrsb = cpool.tile([P, KO, 2, 2, P], fp8)
            nc.sync.dma_start(wdrsb, d_wdr.ap())
            bias_sb = cpool.tile([P, E], f32)
            nc.sync.dma_start(bias_sb, d_bias.ap())
            ident = cpool.tile([P, P], f32)
            nc.sync.dma_start(ident, d_ident.ap())
            negbig = cpool.tile([P, 1], f32)
            nc.vector.memset(negbig, NEG_BIG)
            oidx_sb = cpool.tile([P, N_TILES, TOP_K], u16)
            owgt_sb = cpool.tile([P, N_TILES, TOP_K], f32)

            # HAM warmup: throwaway matmuls while first DMAs are in flight
            scrA = cpool.tile([P, P], f16)
            scrB = cpool.tile([P, 2 * E], f16)
            nc.vector.memset(scrA, 0.0)
            nc.vector.memset(scrB, 0.0)
            for _ in range(N_WARM):
                pw = wpool.tile([P, 2 * E], f32, tag="pw")
                nc.tensor.matmul(pw, lhsT=scrA, rhs=scrB, start=True, stop=True,
                                 skip_group_check=True)

            # per-chunk state carried into the post phase
            pending = []

            def emit_matmuls(ci):
                ct = CHUNKS[ci]
                tl0 = sum(CHUNKS[:ci])
                ctok = ct * P
                xoff16 = tl0 * P * KO
                x16c = xpool.tile([P, KO, ctok], f16, tag="x16")
                nc.sync.dma_start(
                    x16c, d_x16.ap()[xoff16 * P:(xoff16 + ctok * KO) * P]
                    .rearrange("(p ko t) -> p ko t", p=P, ko=KO))
                xdrc = xpool.tile([P, KO, 2, ctok], fp8, tag="xdr")
                nc.sync.dma_start(
                    xdrc, d_xdr.ap()[xoff16 * 2 * P:(xoff16 + ctok * KO) * 2 * P]
                    .rearrange("(p ko two t) -> p ko two t", p=P, ko=KO, two=2))

                lgs = []        # (eh, tb_off, tb_n, lg_tile)
                for eh in range(2):
                    for (tb0, tbn) in _tb_blocks(ct):
                        ntok = tbn * P
                        tsl = slice(tb0 * P, tb0 * P + ntok)
                        psA = ppool.tile([P, ntok], f32, tag="psA")
                        for k in range(KO):
                            nc.tensor.matmul(
                                psA, lhsT=w16sb[:, k, eh, :], rhs=x16c[:, k, tsl],
                                start=(k == 0), stop=(k == KO - 1))
                        psC = ppool.tile([P, ntok], f32, tag="psC")
                        for k in range(KO):
                            nc.tensor.matmul(
                                psC, lhsT=wdrsb[:, k, eh, :, :],
                                rhs=xdrc[:, k, :, tsl],
                                perf_mode=PM.DoubleRow,
                                start=(k == 0), stop=(k == KO - 1))
                        # main psum -> SBUF (ACT), then fold with corr psum
                        lga = lpool.tile([P, ntok], f32, tag="lga")
                        nc.scalar.activation(lga, psA, ACTF.Copy)
                        lg = lpool.tile([P, ntok], f32, tag="lg")
                        nc.vector.scalar_tensor_tensor(
                            lg, psC, 1.0 / SC_CORR, lga,
                            op0=ALU.mult, op1=ALU.add)
                        lgs.append((eh, tb0, tbn, lg))
                return (ci, ct, tl0, lgs)

            def emit_post(state):
                ci, ct, tl0, lgs = state
                sb_st = spool.tile([P, ct, E], f32, tag="sb")
                # transpose each 128-token block back to [token, expert];
                # both expert halves share one PSUM tile -> one sigmoid/tile
                lg_of = {}
                for (eh, tb0, tbn, lg) in lgs:
                    for j in range(tbn):
                        lg_of[(eh, tb0 + j)] = (lg, j)
                for tt in range(ct):
                    pt = tpool.tile([P, 2, P], f32, tag="pt")
                    for eh in range(2):
                        lg, j = lg_of[(eh, tt)]
                        nc.tensor.transpose(pt[:, eh, :],
                                            lg[:, j * P:(j + 1) * P], ident)
                    nc.scalar.activation(sb_st[:, tt, :], pt, ACTF.Sigmoid)

                zap_st = spool.tile([P, ct, E], f32, tag="zap")
                msf_st = spool.tile([P, ct, E], f32, tag="msf")
                t1g = mpool.tile([P, ct, N_GROUP], f32, tag="t1g")
                t2g = mpool.tile([P, ct, N_GROUP], f32, tag="t2g")
                gs = mpool.tile([P, ct, N_GROUP], f32, tag="gs")
                cc = mpool.tile([P, ct, N_GROUP, N_GROUP], f32, tag="cc")
                c8 = mpool.tile([P, ct, N_GROUP], f32, tag="c8")
                madd = mpool.tile([P, ct, N_GROUP], f32, tag="madd")

                # scores_for_choice = sigma + bias
                nc.vector.tensor_add(
                    sb_st, sb_st, bias_sb[:, None, :].to_broadcast([P, ct, E]))

                # group top-2
                sb4 = sb_st.rearrange("p t (g e) -> p t g e", g=N_GROUP)
                nc.vector.tensor_reduce(out=t1g, in_=sb4, axis=AX.X, op=ALU.max)
                for j in range(ct):
                    nc.vector.match_replace(
                        out=zap_st[:, j, :], in_to_replace=t1g[:, j, :],
                        in_values=sb_st[:, j, :], imm_value=NEG_BIG)
                nc.vector.tensor_reduce(
                    out=t2g, in_=zap_st.rearrange("p t (g e) -> p t g e", g=N_GROUP),
                    axis=AX.X, op=ALU.max)
                nc.vector.tensor_add(gs, t1g, t2g)

                # group rank -> additive mask
                nc.vector.tensor_tensor(
                    out=cc,
                    in0=gs[:, :, None, :].to_broadcast([P, ct, N_GROUP, N_GROUP]),
                    in1=gs[:, :, :, None].to_broadcast([P, ct, N_GROUP, N_GROUP]),
                    op=ALU.is_gt)
                nc.vector.tensor_reduce(out=c8, in_=cc, axis=AX.X, op=ALU.add)
                nc.vector.scalar_tensor_tensor(
                    madd, c8, float(TOPK_GROUP) - 0.5,
                    negbig[:, :, None].to_broadcast([P, ct, N_GROUP]),
                    op0=ALU.is_gt, op1=ALU.mult)
                nc.vector.tensor_add(
                    msf_st.rearrange("p t (g e) -> p t g e", g=N_GROUP),
                    sb4,
                    madd[:, :, :, None].to_broadcast([P, ct, N_GROUP, GSIZE]))

                # top-8: raw score values + indices (epilogue on host)
                for j in range(ct):
                    tl = tl0 + j
                    nc.vector.max(out=owgt_sb[:, tl, :], in_=msf_st[:, j, :])
                    nc.vector.max_index(out=oidx_sb[:, tl, :],
                                        in_max=owgt_sb[:, tl, :],
                                        in_values=msf_st[:, j, :])
                ssl = slice(tl0, tl0 + ct)
                nc.scalar.dma_start(d_oidx.ap()[:, ssl, :], oidx_sb[:, ssl, :])
                nc.scalar.dma_start(d_owgt.ap()[:, ssl, :], owgt_sb[:, ssl, :])

            prev = None
            for ci in range(len(CHUNKS)):
                state = emit_matmuls(ci)
                if prev is not None:
                    emit_post(prev)
                prev = state
            emit_post(prev)

    nc.compile()
    return nc


def _get_nc(mode):
    if mode not in _CACHED_NC:
        _CACHED_NC[mode] = build_kernel(mode)
    return _CACHED_NC[mode]


def _pack_x(a):
    """[H, T_CORE] (any dtype) -> packed 1D per-chunk-contiguous blocks
    with [p, ko, t] = a[ko*P+p, tl0*P+t]."""
    arr = a.reshape(KO, P, T_CORE)
    blocks = []
    tl0 = 0
    for ct in CHUNKS:
        blocks.append(np.ascontiguousarray(
            arr[:, :, tl0 * P:(tl0 + ct) * P].transpose(1, 0, 2)).reshape(-1))
        tl0 += ct
    return np.concatenate(blocks)


def _pack_x2(a0, a1):
    """two [H, T_CORE] pieces -> packed 1D with [p, ko, {a0,a1}, t] blocks."""
    s0 = a0.reshape(KO, P, T_CORE)
    s1 = a1.reshape(KO, P, T_CORE)
    blocks = []
    tl0 = 0
    for ct in CHUNKS:
        sl = slice(tl0 * P, (tl0 + ct) * P)
        b = np.stack([s0[:, :, sl], s1[:, :, sl]], axis=2)   # [KO, P, 2, ctok]
        blocks.append(np.ascontiguousarray(b.transpose(1, 0, 2, 3)).reshape(-1))
        tl0 += ct
    return np.concatenate(blocks)


def kernel(hidden_states, w, e_score_correction_bias, mode=MATMUL_MODE):
    import ml_dtypes
    f8 = ml_dtypes.float8_e4m3

    hidden_states = np.asarray(hidden_states)
    w = np.asarray(w)
    bias_f = np.asarray(e_score_correction_bias, np.float32)
    T = hidden_states.shape[0] * hidden_states.shape[1]
    assert T == N_CORES * T_CORE
    x2 = np.ascontiguousarray(hidden_states.reshape(T, H).astype(np.float32))
    xT = np.ascontiguousarray(x2.T)                         # [H, T]
    wT = np.ascontiguousarray(np.asarray(w, np.float32).T)  # [H, E]

    # x pieces
    x16 = xT.astype(np.float16)
    xl = xT - x16.astype(np.float32)
    xl8s = (xl * 2.0**13).astype(f8)
    x8 = xT.astype(f8)

    # w pieces (stationary): [P, KO, eh, e']
    w16 = wT.astype(np.float16)
    wl = wT - w16.astype(np.float32)
    w8s = (wT * 2.0**5).astype(f8)
    wl8s = (wl * 2.0**18).astype(f8)

    def pkw(a):  # [H, E] -> [P, KO, 2, P]
        return np.ascontiguousarray(
            np.asarray(a).reshape(KO, P, 2, P).transpose(1, 0, 2, 3))

    w16p = pkw(w16)
    wdrp = np.ascontiguousarray(np.stack(
        [pkw(np.asarray(w8s, np.float32)), pkw(np.asarray(wl8s, np.float32))],
        axis=3)).astype(f8)                                  # [P, KO, 2, 2, P]

    bias_rep = np.ascontiguousarray(np.repeat(bias_f[None, :], P, 0))
    ident = np.eye(P, dtype=np.float32)

    nc = _get_nc(mode)
    in_maps = []
    for c in range(N_CORES):
        csl = slice(c * T_CORE, (c + 1) * T_CORE)
        m = {"x16p": _pack_x(x16[:, csl]),
             "xdrp": _pack_x2(np.asarray(xl8s[:, csl], np.float32),
                              np.asarray(x8[:, csl], np.float32)).astype(f8),
             "w16p": w16p, "wdrp": wdrp, "biasrep": bias_rep, "ident": ident}
        in_maps.append(m)

    res = run_bass_kernel_spmd(nc, in_maps, core_ids=list(range(N_CORES)))

    idx_parts, wgt_parts = [], []
    for c in range(N_CORES):
        r = res.results[c]
        idx_parts.append(r["oidx"].transpose(1, 0, 2).reshape(T_CORE, TOP_K))
        wgt_parts.append(r["owgt"].transpose(1, 0, 2).reshape(T_CORE, TOP_K))
    topk_idx = np.concatenate(idx_parts, 0).astype(np.int32)
    v8 = np.concatenate(wgt_parts, 0).astype(np.float32)
    sigma = v8 - bias_f[topk_idx]
    topk_weight = (sigma / sigma.sum(-1, keepdims=True) * SCALING).astype(np.float32)
    return topk_idx, topk_weight
